# revision 1
# baseline (speedup 1.0000x reference)
"""Trainium2 Bass kernel for GQA attention forward (B=2, S=2048, D=2048,
16 q-heads / 4 kv-heads, head_dim=128, RoPE, causal).

Sharding: 8 cores = 2 (batch) x 4 (kv-head groups).  Each core computes its
batch's attention for one kv-head group (4 q-heads + 1 kv head) and a
row-parallel partial of the output projection; the host sums the 4 bf16
partials per batch.

Design (all phases software-pipelined over four 512-row s-chunks):
  * q/k projections are emitted directly in [e, s] (transposed) form
    (lhsT = weight tile, rhs = xT tile) -- no PE transposes anywhere.
  * RoPE runs on DVE in the transposed layout via partition-half swaps.
  * Scores keep keys in partitions / queries free, so exp output feeds the
    PV matmul directly.  Only the 128-wide sub-diagonal block is masked
    (columns are realigned so it is always the first written block).
  * The softmax denominator is a bf16 pair tree on DVE over the exp tiles,
    then a 128-partition sum + reciprocal broadcast on the idle Pool
    engine (partition_all_reduce) -- the tensor engine never touches it.
  * Emission order is A(c) | B(*, c) with the previous chunk's output
    projection C(c-1) interleaved after each head of B(c): C's matmuls
    fill the PE gaps where B is exp-throughput-bound, and its PSUM->SBUF
    copies ride whichever of ACT/DVE has slack in that window.
  * x / weights / RoPE tables stream per-chunk (head-major for wq) so the
    first projection chain starts ~2us in; outputs store as bf16 rows.
PSUM budget is exactly 8 banks: k/q/out-proj share 3, scores 3, v/pv 2.
"""

import sys

if "/opt/trn_rl_repo" not in sys.path:
    sys.path.insert(0, "/opt/trn_rl_repo")

import numpy as np
import ml_dtypes

import concourse.bass as bass
import concourse.bass_isa as bass_isa
import concourse.tile as tile
from concourse import mybir

F32 = mybir.dt.float32
F32R = mybir.dt.float32r
BF16 = mybir.dt.bfloat16

# Full-problem constants (per reference).
B, S, DIM = 2, 2048, 2048
N_HEADS, N_KV_HEADS, HEAD_DIM = 16, 4, 128
N_GROUPS = N_KV_HEADS          # tensor-parallel groups
HQ = N_HEADS // N_KV_HEADS     # q heads per group
NEG = -1e30


def build_attention_core(nc, S=S, D=DIM, HQ=HQ, HD=HEAD_DIM, CHUNK=512):
    n_st = S // 128        # s tiles
    n_dt = D // 128        # d tiles
    n_ch = S // CHUNK      # s chunks
    kpc = CHUNK // 128     # k-tiles per chunk
    n_dc = D // CHUNK      # d chunks (phase C)
    spc = CHUNK // 128     # s-tiles per chunk
    IQ = HQ * HD

    x_d = nc.dram_tensor("xT", [128, n_dt, S], BF16, kind="ExternalInput")
    wqT_d = nc.dram_tensor("wqT", [128, HQ, n_dt, HD], BF16,
                           kind="ExternalInput")
    wkvT_d = nc.dram_tensor("wkvT", [128, n_dt, 2 * HD], BF16, kind="ExternalInput")
    woT_d = nc.dram_tensor("woT", [128, IQ // 128, D], BF16, kind="ExternalInput")
    t1_d = nc.dram_tensor("t1", [128, S], BF16, kind="ExternalInput")
    t2_d = nc.dram_tensor("t2", [128, S], BF16, kind="ExternalInput")
    masks_d = nc.dram_tensor("masks", [128, 128], F32, kind="ExternalInput")
    out_d = nc.dram_tensor("out_partial", [S, D], BF16, kind="ExternalOutput")

    scale = float(HD) ** -0.5

    with tile.TileContext(nc) as tc:
        with (
            tc.tile_pool(name="persist", bufs=1) as persist,
            tc.tile_pool(name="xin", bufs=1) as xin_pool,
            tc.tile_pool(name="rope", bufs=4) as rope_pool,
            tc.tile_pool(name="expt", bufs=10) as expt_pool,
            tc.tile_pool(name="acc", bufs=3) as acc_pool,
            tc.tile_pool(name="pairs", bufs=4) as pair_pool,
            tc.tile_pool(name="recip", bufs=3) as rec_pool,
            tc.tile_pool(name="outsb", bufs=4) as outsb_pool,
            # PSUM: 8 banks total
            tc.tile_pool(name="ps_a", bufs=3, space="PSUM") as psa_pool,   # 3
            tc.tile_pool(name="ps_s", bufs=3, space="PSUM") as pss_pool,   # 3
            tc.tile_pool(name="ps_o", bufs=2, space="PSUM") as pso_pool,   # 2
        ):
            # ---------------- weights + constants ---------------------------
            wq_sb = persist.tile([128, HQ, n_dt, HD], BF16)
            wkv_sb = persist.tile([128, n_dt, 2 * HD], BF16)
            t1_sb = persist.tile([128, S], BF16)
            t2_sb = persist.tile([128, S], BF16)
            c0 = slice(0, CHUNK)
            for g in range(n_dt // 4):
                gs = slice(g * 4, (g + 1) * 4)
                nc.scalar.dma_start(out=wkv_sb[:, gs, :], in_=wkvT_d[:, gs, :])
                if g == 0:
                    # chunk-0 RoPE tables early: k-rope needs them ~12us in
                    nc.scalar.dma_start(out=t1_sb[:, c0], in_=t1_d[:, c0])
                    nc.scalar.dma_start(out=t2_sb[:, c0], in_=t2_d[:, c0])
            for h in range(HQ):
                # head-major: q-chain h can start as soon as its slab lands
                nc.scalar.dma_start(out=wq_sb[:, h, :, :], in_=wqT_d[:, h, :, :])
            masks_sb = persist.tile([128, 128], F32)
            nc.scalar.dma_start(out=masks_sb, in_=masks_d[:])
            rest = slice(CHUNK, S)
            nc.scalar.dma_start(out=t1_sb[:, rest], in_=t1_d[:, rest])
            nc.scalar.dma_start(out=t2_sb[:, rest], in_=t2_d[:, rest])
            woT_sb = persist.tile([128, IQ // 128, D], BF16)

            # x streamed per s-chunk on the sync queue
            x_ch = []
            for c in range(n_ch):
                xt = xin_pool.tile([128, n_dt, CHUNK], BF16, tag=f"x{c % 2}",
                                   name=f"x{c}")
                c_sl = slice(c * CHUNK, (c + 1) * CHUNK)
                ng = 2 if c == 0 else 4
                for g4 in range(n_dt // ng):
                    nc.sync.dma_start(
                        out=xt[:, g4 * ng:(g4 + 1) * ng, :],
                        in_=x_d[:, g4 * ng:(g4 + 1) * ng, c_sl],
                    )
                x_ch.append(xt)
                if c == 1:
                    nc.sync.dma_start(out=woT_sb, in_=woT_d[:])

            # persistent activations
            qT_sb = persist.tile([128, HQ, S], BF16)    # [e, h, s]
            kT_sb = persist.tile([128, S], BF16)        # [e, s]
            v_sb = persist.tile([128, n_st, HD], BF16)  # [s_in_tile, s_tile, e]
            oT_sb = persist.tile([128, HQ, S], BF16)    # [e, h, s]

            # deferred per-(h,c) normalization tail (keeps PE from stalling
            # on the DVE denominator chain)
            norm_pending = [None]

            def emit_norm():
                acc_, ps_o_, h_, c_ = norm_pending[0]
                norm_pending[0] = None
                # softmax denominator: 128-partition sum of the bf16 pair
                # tree, broadcast to all partitions, on the idle Pool engine
                sum_sb = rec_pool.tile([128, CHUNK], F32, tag="sum_sb")
                nc.gpsimd.partition_all_reduce(
                    sum_sb, acc_, channels=128, reduce_op=bass_isa.ReduceOp.add
                )
                rec_sb = rec_pool.tile([128, CHUNK], F32, tag="rec_sb")
                nc.vector.reciprocal_approx_fast(rec_sb, sum_sb)
                nc.vector.tensor_mul(
                    oT_sb[:, h_, c_ * CHUNK:(c_ + 1) * CHUNK], ps_o_, rec_sb
                )

            def emit_out_tile(c, sj, last=False):
                """Phase C for s-tile sj of chunk c: one 128-row output slab.
                Interleaved into B(c+1)'s head loop: its matmuls fill the PE
                gaps where B is exp-throughput-bound, and its PSUM->SBUF
                copies run on DVE (ACT is the B-window pacer).  GPSIMD
                cannot access PSUM, so Pool takes no copies."""
                st = c * spc + sj
                row_sb = outsb_pool.tile([128, D], BF16, tag="out_sb")
                for dc in range(n_dc):
                    if last and dc % 2 == 1:
                        ps_d = pss_pool.tile([128, CHUNK], F32, tag="ps_s")
                    else:
                        ps_d = psa_pool.tile([128, CHUNK], F32, tag="ps_a")
                    for it in range(HQ):
                        nc.tensor.matmul(
                            ps_d,
                            oT_sb[:, it, st * 128:(st + 1) * 128],
                            woT_sb[:, it, dc * CHUNK:(dc + 1) * CHUNK],
                            start=(it == 0), stop=(it == HQ - 1),
                        )
                    dst = row_sb[:, dc * CHUNK:(dc + 1) * CHUNK]
                    # engine choice tracks which engine has slack in the
                    # B window this chunk interleaves with (ACT saturates
                    # as c grows; DVE is flatter)
                    if c == 0:
                        use_act = True
                    elif c == 1:
                        use_act = dc % 2 == 0
                    elif c == 2:
                        use_act = False
                    else:
                        use_act = dc % 2 == 0
                    if use_act:
                        nc.scalar.copy(dst, ps_d)
                    else:
                        nc.vector.tensor_copy(dst, ps_d)
                if last and sj == spc - 1:
                    # final tile: split the store so the tail drains as the
                    # copies complete instead of after the whole row
                    for dc in range(n_dc):
                        nc.sync.dma_start(
                            out=out_d[st * 128:(st + 1) * 128,
                                      dc * CHUNK:(dc + 1) * CHUNK],
                            in_=row_sb[:, dc * CHUNK:(dc + 1) * CHUNK],
                        )
                else:
                    nc.sync.dma_start(
                        out=out_d[st * 128:(st + 1) * 128, :], in_=row_sb
                    )

            def rope(dst, src, c):
                """dst[e, s-chunk] = src*t1 + swap_half(src)*t2 (DVE)."""
                c_sl = slice(c * CHUNK, (c + 1) * CHUNK)
                t1c = t1_sb[:, c_sl]
                t2c = t2_sb[:, c_sl]
                m1 = rope_pool.tile([128, CHUNK], F32, tag="m1")
                nc.vector.tensor_mul(m1, src, t1c)
                m2 = rope_pool.tile([128, CHUNK], F32, tag="m2")
                nc.vector.tensor_mul(m2[0:64, :], src[64:128, :], t2c[0:64, :])
                nc.vector.tensor_mul(m2[64:128, :], src[0:64, :], t2c[64:128, :])
                nc.vector.tensor_add(dst, m1, m2)

            for c in range(n_ch):
                c_sl = slice(c * CHUNK, (c + 1) * CHUNK)
                xt = x_ch[c]

                # ======== A(c): projections + RoPE ========
                ps_k = psa_pool.tile([128, CHUNK], F32, tag="ps_a")
                for dt_ in range(n_dt):
                    nc.tensor.matmul(
                        ps_k, wkv_sb[:, dt_, 0:HD], xt[:, dt_, :],
                        start=(dt_ == 0), stop=(dt_ == n_dt - 1),
                    )
                if norm_pending[0] is not None:
                    emit_norm()
                rope(kT_sb[:, c_sl], ps_k, c)

                for h in range(HQ):
                    ps_qh = psa_pool.tile([128, CHUNK], F32, tag="ps_a")
                    for dt_ in range(n_dt):
                        nc.tensor.matmul(
                            ps_qh, wq_sb[:, h, dt_, :], xt[:, dt_, :],
                            start=(dt_ == 0), stop=(dt_ == n_dt - 1),
                        )
                    rope(qT_sb[:, h, c_sl], ps_qh, c)

                # v: natural [s, e] layout, one chain per s-tile; the
                # four chains share one bank from the ps_o rotation
                ps_vt = pso_pool.tile([128, CHUNK], F32, tag="o")
                for sj in range(spc):
                    st = c * spc + sj
                    sj_sl = slice(sj * 128, (sj + 1) * 128)
                    for dt_ in range(n_dt):
                        nc.tensor.matmul(
                            ps_vt[:, sj_sl], xt[:, dt_, sj_sl],
                            wkv_sb[:, dt_, HD:2 * HD],
                            start=(dt_ == 0), stop=(dt_ == n_dt - 1),
                        )
                    nc.scalar.copy(v_sb[:, st, :], ps_vt[:, sj_sl])

                # ======== B(*, c): attention for q-chunk c ========
                for h in range(HQ):
                    ps_o = pso_pool.tile([128, CHUNK], F32, tag="o")
                    n_kj = (c + 1) * kpc
                    acc = acc_pool.tile([128, CHUNK], BF16, tag="acc")
                    pend_pv = []
                    stash_exp = [None]
                    stash_pair = [None]
                    n_acc = [0]

                    def flush_pv():
                        pe, pj, poff = pend_pv.pop(0)
                        nc.tensor.matmul(
                            ps_o[:, poff:], v_sb[:, pj, :], pe,
                            start=(pj == 0), stop=(pj == n_kj - 1),
                        )

                    for kj in range(n_kj):
                        off = max(0, (kj - c * kpc)) * 128
                        w = CHUNK - off
                        ps_s = pss_pool.tile([128, CHUNK], F32, tag="ps_s")
                        nc.tensor.matmul(
                            ps_s[:, 0:w],
                            kT_sb[:, kj * 128:(kj + 1) * 128],
                            qT_sb[:, h, c * CHUNK + off:(c + 1) * CHUNK],
                            start=True, stop=True,
                        )
                        if kj == 1 and norm_pending[0] is not None:
                            emit_norm()
                        if kj >= c * kpc:
                            # causal mask: ps_s column i holds q-position
                            # off+i, so the partial 128-wide diagonal block
                            # is always the first 128 written columns
                            nc.vector.tensor_add(
                                ps_s[:, 0:128], ps_s[:, 0:128], masks_sb,
                            )
                        expT = expt_pool.tile([128, CHUNK], BF16, tag="expT")
                        if off > 0:
                            # exp output is realigned to q-in-chunk columns;
                            # zero the fully-masked leading columns so the
                            # denominator tree can run full-width
                            nc.gpsimd.memset(expT[:, 0:off], 0.0)
                        nc.scalar.activation(
                            expT[:, off:], ps_s[:, 0:w],
                            mybir.ActivationFunctionType.Exp,
                            scale=scale,
                        )
                        pend_pv.append((expT[:, off:], kj, off))
                        if len(pend_pv) > 2:
                            flush_pv()
                        # denominator: bf16 pair tree on DVE (full width --
                        # masked regions of expT are zeroed above)
                        if kj % 2 == 0:
                            stash_exp[0] = expT
                        else:
                            pr = pair_pool.tile([128, CHUNK], BF16, tag="pair")
                            nc.vector.tensor_add(pr, stash_exp[0], expT)
                            stash_exp[0] = None
                            if n_acc[0] == 0 and stash_pair[0] is None:
                                stash_pair[0] = pr
                            elif n_acc[0] == 0:
                                nc.vector.tensor_add(acc, stash_pair[0], pr)
                                stash_pair[0] = None
                                n_acc[0] = 1
                            else:
                                nc.vector.tensor_add(acc, acc, pr)
                                n_acc[0] += 1
                    while pend_pv:
                        flush_pv()
                    # n_kj is always >= 4 so at least two pairs were formed
                    # and acc is initialized by the second pair.
                    assert n_acc[0] >= 1
                    norm_pending[0] = (acc, ps_o, h, c)
                    if c > 0:
                        emit_out_tile(c - 1, h)

            emit_norm()
            for sj in range(spc):
                emit_out_tile(n_ch - 1, sj, last=True)

    return nc


# ---------------------------------------------------------------------------
# Host-side prep


_ROPE_PERM = np.concatenate([np.arange(0, HEAD_DIM, 2), np.arange(1, HEAD_DIM, 2)])


def _prep_tables(freq_cis):
    """RoPE tables in [e, s] permuted-half layout.

    rot[0:64]   = q[0:64]*cos   + q[64:128]*(-sin)
    rot[64:128] = q[64:128]*cos + q[0:64]*sin
    """
    fc = np.asarray(freq_cis, dtype=np.float32)
    A = fc[:, :, 0, 0]    # cos  [S, 64]
    Bm = fc[:, :, 0, 1]   # -sin
    C = fc[:, :, 1, 0]    # sin
    Dm = fc[:, :, 1, 1]   # cos
    t1 = np.concatenate([A, Dm], axis=1).T    # [128, S]
    t2 = np.concatenate([Bm, C], axis=1).T
    return (_bf16(t1), _bf16(t2))


def _prep_masks():
    q = np.arange(128)[None, :]
    p = np.arange(128)[:, None]
    return np.where(q >= p, np.float32(0.0), np.float32(NEG))


def _perm_head_rows(w):
    """Permute rows within each 128-row head block: evens first, odds second."""
    nh = w.shape[0] // HEAD_DIM
    return np.ascontiguousarray(
        w.reshape(nh, HEAD_DIM, -1)[:, _ROPE_PERM, :].reshape(w.shape)
    )


def _bf16(a):
    return np.ascontiguousarray(a.astype(ml_dtypes.bfloat16))


def _pmajor(a):
    """[T*128, F...] -> [128, T, F...] partition-major layout."""
    t = a.shape[0] // 128
    return np.ascontiguousarray(
        a.reshape(t, 128, *a.shape[1:]).swapaxes(0, 1)
    )


def make_core_inputs(x, freq_cis, wq, wk, wv, wo):
    """Build the 8 per-core input maps."""
    x = np.asarray(x, np.float32)
    wq = np.asarray(wq, np.float32)
    wk = np.asarray(wk, np.float32)
    wv = np.asarray(wv, np.float32)
    wo = np.asarray(wo, np.float32)
    t1, t2 = _prep_tables(freq_cis)
    masks = _prep_masks()
    IQ = HQ * HEAD_DIM

    in_maps = []
    for core in range(8):
        b, g = divmod(core, N_GROUPS)
        wq_g = _perm_head_rows(wq[g * IQ:(g + 1) * IQ])
        wk_g = _perm_head_rows(wk[g * HEAD_DIM:(g + 1) * HEAD_DIM])
        wv_g = wv[g * HEAD_DIM:(g + 1) * HEAD_DIM]
        # [D, IQ] -> [128, dt, IQ] -> [128, HQ, dt, HD] head-major
        wqT = _pmajor(_bf16(wq_g.T)).reshape(128, 16, HQ, HEAD_DIM)
        wqT = np.ascontiguousarray(wqT.swapaxes(1, 2))
        wkvT = _pmajor(_bf16(np.concatenate([wk_g.T, wv_g.T], axis=1)))
        woT = _pmajor(_bf16(wo[:, g * IQ:(g + 1) * IQ].T))
        in_maps.append({
            "xT": _pmajor(_bf16(x[b].T)),
            "wqT": wqT,
            "wkvT": wkvT,
            "woT": woT,
            "t1": t1,
            "t2": t2,
            "masks": np.ascontiguousarray(masks),
        })
    return in_maps


_CACHED_NC = None


def _get_nc():
    global _CACHED_NC
    if _CACHED_NC is None:
        from concourse import bacc

        nc = bacc.Bacc("TRN2", target_bir_lowering=False, debug=False)
        build_attention_core(nc)
        nc.compile()
        _CACHED_NC = nc
    return _CACHED_NC


def kernel(x, freq_cis, wq, wk, wv, wo):
    from concourse.bass_utils import run_bass_kernel_spmd

    nc = _get_nc()
    in_maps = make_core_inputs(x, freq_cis, wq, wk, wv, wo)
    res = run_bass_kernel_spmd(nc, in_maps, list(range(8)))
    out = np.zeros((B, S, DIM), dtype=np.float32)
    for core in range(8):
        b = core // N_GROUPS
        out[b] += res.results[core]["out_partial"].astype(np.float32)
    return out

